# revision 6
# baseline (speedup 1.0000x reference)
"""Self-contained Trainium2 Bass kernel for the HQNN problem (v2).

Math: the 4-qubit circuit after angle embedding applies a fixed unitary whose
Heisenberg-evolved Z observables are sparse Pauli sums over {I,Y,Z}; each
hybrid layer reduces to tanh -> sin/cos -> a few elementwise products -> small
static matmuls (folded with the next Dense layer). Data-parallel over 8 cores.

v2: fp16 datapath. x is cast fp32->fp16 by a gpsimd DMA into DRAM scratch,
then DMA-transposed (XBAR) straight into SBUF in feature-major layout -- no
PE transposes or PSUM->SBUF copies on the input side. All matmuls are fp16
(1 cyc/row vs 4 for fp32). DVE/ACT passes are 1024 wide; partition shuffles
run on a uint32 view (halves 1x-mode shuffle cost).
"""
import sys
sys.path.insert(0, "/opt/trn_rl_repo")
import itertools
import contextlib
import numpy as np

import concourse.bass as bass
import concourse.bacc as bacc
import concourse.tile as tile
from concourse import mybir
from concourse.bass_utils import run_bass_kernel_spmd
from concourse.masks import make_identity

F32 = mybir.dt.float32
F16 = mybir.dt.float16
U32 = mybir.dt.uint32
PI2 = float(np.pi / 2)
N_CORES = 8
B_TOTAL, D_IN = 524288, 16
B_CORE = B_TOTAL // N_CORES
N_SS = B_CORE // 16384  # supersteps of 16384 rows
CH = 4                  # chunks of 4096 rows per superstep

# ---------------- host-side math ----------------
_I2 = np.eye(2, dtype=complex)
_PY = np.array([[0, -1j], [1j, 0]])
_PZ = np.array([[1, 0], [0, -1]], dtype=complex)
SUPPORTS = [(0, 1, 3), (0, 2, 3), (1, 3), (0, 2)]


def _kron(ms):
    out = np.array([[1.0 + 0j]])
    for m in ms:
        out = np.kron(out, m)
    return out


def _op_on(w, m):
    return _kron([m if v == w else _I2 for v in range(4)])


def _layer_tensors(theta_l):
    U = np.eye(16, dtype=complex)
    for l in range(2):
        for w in range(4):
            c, s = np.cos(theta_l[l, w] / 2), np.sin(theta_l[l, w] / 2)
            U = _op_on(w, np.array([[c, -1j * s], [-1j * s, c]])) @ U
        for w in range(4):
            t = (w + 1) % 4
            C = np.zeros((16, 16), dtype=complex)
            for k in range(16):
                bits = [(k >> (3 - v)) & 1 for v in range(4)]
                if bits[w] == 1:
                    bits[t] ^= 1
                C[sum(b << (3 - v) for v, b in enumerate(bits)), k] = 1
            U = C @ U
    letters = {"I": _I2, "Y": _PY, "Z": _PZ}
    out = []
    for w, sup in enumerate(SUPPORTS):
        H = U.conj().T @ _op_on(w, _PZ) @ U
        T = np.zeros((2,) * len(sup))
        for s in itertools.product("IYZ", repeat=4):
            P = _kron([letters[c] for c in s])
            co = float(np.real(np.trace(P.conj().T @ H) / 16))
            if abs(co) < 1e-10:
                continue
            nz = tuple(v for v in range(4) if s[v] != "I")
            assert set(nz).issubset(set(sup)), f"support {s} w={w}"
            idx, sign = [], 1.0
            ok = True
            for v in sup:
                if s[v] == "I":
                    ok = False
                    break
                idx.append(0 if s[v] == "Y" else 1)
                if s[v] == "Y":
                    sign = -sign
            if not ok:
                assert abs(co) < 1e-10
                continue
            T[tuple(idx)] = sign * co
        out.append(T)
    return out  # C0, C1, B2, B3


def _blockdiag(blk, n):
    K, M = blk.shape
    out = np.zeros((K * n, M * n), dtype=np.float32)
    for i in range(n):
        out[i * K:(i + 1) * K, i * M:(i + 1) * M] = blk
    return out


WMAP = [3, 0, 1, 2, 3, 0, 1, 2]


def host_tensors(theta, W0, b0, W1, b1, W2, b2):
    t = {}
    for i in range(3):
        C0, C1, B2, B3 = _layer_tensors(np.asarray(theta[i], dtype=np.float64))
        A1 = np.zeros((8, 8), dtype=np.float32)
        for a in range(2):
            for c in range(2):
                gi = a * 2 + c
                A1[gi, 1] = C0[a, 0, c]
                A1[gi, 5] = C0[a, 1, c]
                A1[gi, 2] = C1[a, 0, c]
                A1[gi, 6] = C1[a, 1, c]
        A2 = np.zeros((8, 8), dtype=np.float32)
        for b in range(2):
            A2[1 + 4 * b, 3] = B2[b, 0]
            A2[1 + 4 * b, 7] = B2[b, 1]
            A2[2 + 4 * b, 0] = B3[0, b]
            A2[2 + 4 * b, 4] = B3[1, b]
        t[f"lA1_{i}"] = _blockdiag(A1, 16)
        t[f"lA2_{i}"] = _blockdiag(A2, 16)
    D0 = np.zeros((16, 8), dtype=np.float32)
    D0[:, 0:4] = W0
    D0[:, 4:8] = W0
    t["lD0"] = _blockdiag(D0, 8)
    for i, W in [(1, W1), (2, W2)]:
        D = np.zeros((8, 8), dtype=np.float32)
        for k in range(8):
            for j in range(4):
                D[k, j] = W[WMAP[k], j]
                D[k, j + 4] = W[WMAP[k], j]
        t[f"lD{i}"] = _blockdiag(D, 16)
    PO = np.zeros((8, 4), dtype=np.float32)
    for k in range(8):
        PO[k, WMAP[k]] = 1.0
    t["lPO"] = _blockdiag(PO, 16)
    consts = np.zeros((128, 4), dtype=np.float32)
    for i, b in enumerate((b0, b1, b2)):
        consts[:, i] = np.tile(np.tile(np.asarray(b, np.float32), 2), 16)
    consts[:, 3] = np.tile([0., 0., 0., 0., PI2, PI2, PI2, PI2], 16)
    t["consts"] = consts
    for name in list(t):
        if name != "consts":
            t[name] = t[name].astype(np.float16)
    return t


# ---------------- device kernel ----------------
MASK_A = [0, 0, 4, 4, 0, 0, 0, 0]
MASK_B = [3, 7, 3, 7, 0, 0, 0, 0]
W16_NAMES = ["lD0", "lD1", "lD2", "lA1_0", "lA2_0", "lA1_1", "lA2_1",
             "lA1_2", "lA2_2", "lPO"]
W_NAMES = W16_NAMES + ["consts"]
W_COLS = {"lD0": 64, "lD1": 128, "lD2": 128, "lA1_0": 128, "lA2_0": 128,
          "lA1_1": 128, "lA2_1": 128, "lA1_2": 128, "lA2_2": 128,
          "lPO": 64, "consts": 4}


def build_kernel(tc, x, out, wins):
    nc = tc.nc
    shufA = [8 * t_ + MASK_A[j] for t_ in range(4) for j in range(8)]
    shufB = [8 * t_ + MASK_B[j] for t_ in range(4) for j in range(8)]
    with contextlib.ExitStack() as ctx:
        wpool = ctx.enter_context(tc.tile_pool(name="w", bufs=1))
        dram = ctx.enter_context(tc.tile_pool(name="dram", bufs=2, space="DRAM"))
        xkp = ctx.enter_context(tc.tile_pool(name="xk", bufs=8))
        work = ctx.enter_context(tc.tile_pool(name="work", bufs=3))
        outp = ctx.enter_context(tc.tile_pool(name="outp", bufs=2))
        ps_mm = ctx.enter_context(tc.tile_pool(name="ps_mm", bufs=5, space="PSUM"))
        ps_po = ctx.enter_context(tc.tile_pool(name="ps_po", bufs=2, space="PSUM"))
        ps_ob = ctx.enter_context(tc.tile_pool(name="ps_ob", bufs=1, space="PSUM"))

        wt = {}
        for name in W16_NAMES:
            wtile = wpool.tile([128, W_COLS[name]], F16, tag=name)
            nc.sync.dma_start(wtile[:], wins[name][:, :])
            wt[name] = wtile
        ctile = wpool.tile([128, 4], F32, tag="consts")
        nc.sync.dma_start(ctile[:], wins["consts"][:, :])
        ident = wpool.tile([128, 128], F32, tag="ident")
        make_identity(nc, ident)
        lA1 = [wt["lA1_0"], wt["lA1_1"], wt["lA1_2"]]
        lA2 = [wt["lA2_0"], wt["lA2_1"], wt["lA2_2"]]
        lD = [None, wt["lD1"], wt["lD2"]]

        xin = x.rearrange("(ss k) f -> ss k f", ss=N_SS)
        # out row = ss*16384 + c*4096 + aa*1024 + ap*8 + r
        ov = out.rearrange("(ss c aa ap r) w -> ss aa ap c (r w)",
                           ss=N_SS, c=CH, aa=4, ap=128, r=8)

        for ss in range(N_SS):
            x16 = dram.tile([16384, 16], F16, tag="x16")
            nc.gpsimd.dma_start(x16[:], xin[ss])  # fp32 -> fp16 cast in DMA
            x16v = x16[:].rearrange("(c a r) f -> c a (r f)", c=CH, a=512, r=8)
            xks = []
            for c in range(CH):
                xk = xkp.tile([128, 512], F16, tag="xk")
                nc.sync.dma_start(xk[:], x16v[c], transpose=True)
                xks.append(xk)

            # two independent 512-wide half-pipelines (half h covers chunks
            # 2h, 2h+1 = 8192 rows) so engines stay densely fed
            pre = [None, None]
            for h in range(2):
                pre[h] = ps_mm.tile([128, 512], F32, tag="mm", name="pre")
                nc.tensor.matmul(pre[h][0:64, :], wt["lD0"][:], xks[2 * h][:],
                                 start=True, stop=True)
                nc.tensor.matmul(pre[h][64:128, :], wt["lD0"][:], xks[2 * h + 1][:],
                                 start=True, stop=True)

            vin = [None, None]
            for li in range(3):
                for h in range(2):
                    if li > 0:
                        pre[h] = ps_mm.tile([128, 512], F32, tag="mm", name="pre")
                        nc.tensor.matmul(pre[h][:, :], lD[li][:], vin[h][:],
                                         start=True, stop=True)
                    h8 = work.tile([128, 512], F16, tag="h8")
                    nc.scalar.activation(h8[:], pre[h][:],
                                         mybir.ActivationFunctionType.Tanh,
                                         bias=ctile[:, li:li + 1], scale=1.0)
                    trig = work.tile([128, 512], F16, tag="trig")
                    nc.scalar.activation(trig[:], h8[:],
                                         mybir.ActivationFunctionType.Sin,
                                         bias=ctile[:, 3:4], scale=1.0)
                    ga = work.tile([128, 512], F16, tag="ga")
                    gb = work.tile([128, 512], F16, tag="gb")
                    nc.vector.stream_shuffle(ga[:].bitcast(U32),
                                             trig[:].bitcast(U32), shufA)
                    nc.vector.stream_shuffle(gb[:].bitcast(U32),
                                             trig[:].bitcast(U32), shufB)
                    g = work.tile([128, 512], F16, tag="g")
                    nc.gpsimd.tensor_mul(g[:], ga[:], gb[:])
                    r1 = ps_mm.tile([128, 512], F32, tag="mm")
                    nc.tensor.matmul(r1[:], lA1[li][:], g[:], start=True, stop=False)
                    nc.tensor.matmul(r1[:], lA2[li][:], trig[:], start=False, stop=True)
                    v = work.tile([128, 512], F16, tag="v")
                    nc.vector.tensor_mul(v[:], trig[:], r1[:])
                    vin[h] = v

            so = [None, None]
            for h in range(2):
                po = ps_po.tile([64, 512], F32, tag="po")
                nc.tensor.matmul(po[:], wt["lPO"][:], vin[h][:], start=True, stop=True)
                so[h] = outp.tile([64, 512], F32, tag="so", name="so")
                nc.scalar.copy(so[h][:], po[:])
            ob = ps_ob.tile([128, 512], F32, tag="ob")
            for k in range(8):
                h, kk = k >> 2, k & 3
                nc.tensor.transpose(ob[:, k * 64:(k + 1) * 64],
                                    so[h][:, kk * 128:(kk + 1) * 128],
                                    ident[0:64, 0:64])
            sob = outp.tile([128, 512], F32, tag="sob")
            nc.vector.tensor_copy(sob[:], ob[:])
            for k in range(8):
                chi, aa = k >> 2, k & 3
                nc.scalar.dma_start(ov[ss, aa, :, 2 * chi:2 * chi + 2],
                                    sob[:, k * 64:(k + 1) * 64])


# Force Tanh/Sin into a single resident ACT table set (silu_and_others holds
# both) so the table-load pass doesn't thrash between per-func sets. Dict
# order/indices are preserved so act_func_set_id stays consistent.
from concourse import hw_specs as _hw_specs
import concourse.bacc as _bacc_mod
_orig_get_tables = _hw_specs.get_activation_tables

def _patched_get_tables(arch):
    tabs = _orig_get_tables(arch)
    out = {}
    for name, s in tabs.items():
        s2 = set(s)
        if name != "silu_and_others":
            s2.discard(mybir.ActivationFunctionType.Tanh)
            s2.discard(mybir.ActivationFunctionType.Sin)
        out[name] = s2
    return out

_hw_specs.get_activation_tables = _patched_get_tables
for _mod in (_bacc_mod,):
    if hasattr(_mod, "get_activation_tables"):
        _mod.get_activation_tables = _patched_get_tables


_CACHE = {}


def _get_compiled():
    if "nc" in _CACHE:
        return _CACHE["nc"], _CACHE["tiles"]
    nc = bacc.Bacc("TRN2", target_bir_lowering=False, debug=False,
                   num_devices=N_CORES)
    x_ap = nc.dram_tensor("x", [B_CORE, D_IN], F32, kind="ExternalInput").ap()
    out_ap = nc.dram_tensor("out", [B_CORE, 4], F32, kind="ExternalOutput").ap()
    wins = {}
    for name in W16_NAMES:
        wins[name] = nc.dram_tensor(name, [128, W_COLS[name]], F16,
                                    kind="ExternalInput").ap()
    wins["consts"] = nc.dram_tensor("consts", [128, 4], F32,
                                    kind="ExternalInput").ap()
    with tile.TileContext(nc) as tc:
        build_kernel(tc, x_ap, out_ap, wins)
    nc.compile()
    _CACHE["nc"] = nc
    _CACHE["tiles"] = None
    return nc, None


def kernel(x, theta, W0, b0, W1, b1, W2, b2):
    x = np.ascontiguousarray(np.asarray(x, dtype=np.float32))
    wt = host_tensors(np.asarray(theta), np.asarray(W0), np.asarray(b0),
                      np.asarray(W1), np.asarray(b1), np.asarray(W2),
                      np.asarray(b2))
    nc, _ = _get_compiled()
    in_maps = []
    for c in range(N_CORES):
        m = {"x": np.ascontiguousarray(x[c * B_CORE:(c + 1) * B_CORE])}
        for name in W_NAMES:
            m[name] = wt[name]
        in_maps.append(m)
    res = run_bass_kernel_spmd(nc, in_maps, core_ids=list(range(N_CORES)))
    outs = [res.results[c]["out"] for c in range(N_CORES)]
    return np.concatenate(outs, axis=0).astype(np.float32)


# revision 10
# speedup vs baseline: 1.1364x; 1.1364x over previous
"""Self-contained Trainium2 Bass kernel for the HQNN problem (v2).

Math: the 4-qubit circuit after angle embedding applies a fixed unitary whose
Heisenberg-evolved Z observables are sparse Pauli sums over {I,Y,Z}; each
hybrid layer reduces to tanh -> sin/cos -> a few elementwise products -> small
static matmuls (folded with the next Dense layer). Data-parallel over 8 cores.

v2: fp16 datapath. x is cast fp32->fp16 by a gpsimd DMA into DRAM scratch,
then DMA-transposed (XBAR) straight into SBUF in feature-major layout -- no
PE transposes or PSUM->SBUF copies on the input side. All matmuls are fp16
(1 cyc/row vs 4 for fp32). DVE/ACT passes are 1024 wide; partition shuffles
run on a uint32 view (halves 1x-mode shuffle cost).
"""
import sys
sys.path.insert(0, "/opt/trn_rl_repo")
import itertools
import contextlib
import numpy as np

import concourse.bass as bass
import concourse.bacc as bacc
import concourse.tile as tile
from concourse import mybir
from concourse.bass_utils import run_bass_kernel_spmd
from concourse.masks import make_identity

F32 = mybir.dt.float32
F16 = mybir.dt.bfloat16
U16 = mybir.dt.uint16
U32 = mybir.dt.uint32
PI2 = float(np.pi / 2)
N_CORES = 8
B_TOTAL, D_IN = 524288, 16
B_CORE = B_TOTAL // N_CORES
N_SS = B_CORE // 16384  # supersteps of 16384 rows
CH = 4                  # chunks of 4096 rows per superstep

# ---------------- host-side math ----------------
_I2 = np.eye(2, dtype=complex)
_PY = np.array([[0, -1j], [1j, 0]])
_PZ = np.array([[1, 0], [0, -1]], dtype=complex)
SUPPORTS = [(0, 1, 3), (0, 2, 3), (1, 3), (0, 2)]


def _kron(ms):
    out = np.array([[1.0 + 0j]])
    for m in ms:
        out = np.kron(out, m)
    return out


def _op_on(w, m):
    return _kron([m if v == w else _I2 for v in range(4)])


def _layer_tensors(theta_l):
    U = np.eye(16, dtype=complex)
    for l in range(2):
        for w in range(4):
            c, s = np.cos(theta_l[l, w] / 2), np.sin(theta_l[l, w] / 2)
            U = _op_on(w, np.array([[c, -1j * s], [-1j * s, c]])) @ U
        for w in range(4):
            t = (w + 1) % 4
            C = np.zeros((16, 16), dtype=complex)
            for k in range(16):
                bits = [(k >> (3 - v)) & 1 for v in range(4)]
                if bits[w] == 1:
                    bits[t] ^= 1
                C[sum(b << (3 - v) for v, b in enumerate(bits)), k] = 1
            U = C @ U
    letters = {"I": _I2, "Y": _PY, "Z": _PZ}
    out = []
    for w, sup in enumerate(SUPPORTS):
        H = U.conj().T @ _op_on(w, _PZ) @ U
        T = np.zeros((2,) * len(sup))
        for s in itertools.product("IYZ", repeat=4):
            P = _kron([letters[c] for c in s])
            co = float(np.real(np.trace(P.conj().T @ H) / 16))
            if abs(co) < 1e-10:
                continue
            nz = tuple(v for v in range(4) if s[v] != "I")
            assert set(nz).issubset(set(sup)), f"support {s} w={w}"
            idx, sign = [], 1.0
            ok = True
            for v in sup:
                if s[v] == "I":
                    ok = False
                    break
                idx.append(0 if s[v] == "Y" else 1)
                if s[v] == "Y":
                    sign = -sign
            if not ok:
                assert abs(co) < 1e-10
                continue
            T[tuple(idx)] = sign * co
        out.append(T)
    return out  # C0, C1, B2, B3


def _blockdiag(blk, n):
    K, M = blk.shape
    out = np.zeros((K * n, M * n), dtype=np.float32)
    for i in range(n):
        out[i * K:(i + 1) * K, i * M:(i + 1) * M] = blk
    return out


WMAP = [3, 0, 1, 2, 3, 0, 1, 2]


def _to_bf16(a):
    import ml_dtypes
    return a.astype(ml_dtypes.bfloat16)


def host_tensors(theta, W0, b0, W1, b1, W2, b2):
    t = {}
    for i in range(3):
        C0, C1, B2, B3 = _layer_tensors(np.asarray(theta[i], dtype=np.float64))
        A1 = np.zeros((8, 8), dtype=np.float32)
        for a in range(2):
            for c in range(2):
                gi = a * 2 + c
                A1[gi, 1] = C0[a, 0, c]
                A1[gi, 5] = C0[a, 1, c]
                A1[gi, 2] = C1[a, 0, c]
                A1[gi, 6] = C1[a, 1, c]
        A2 = np.zeros((8, 8), dtype=np.float32)
        for b in range(2):
            A2[1 + 4 * b, 3] = B2[b, 0]
            A2[1 + 4 * b, 7] = B2[b, 1]
            A2[2 + 4 * b, 0] = B3[0, b]
            A2[2 + 4 * b, 4] = B3[1, b]
        t[f"lA1_{i}"] = _blockdiag(A1, 16)
        t[f"lA2_{i}"] = _blockdiag(A2, 16)
    D0 = np.zeros((16, 8), dtype=np.float32)
    D0[:, 0:4] = W0
    D0[:, 4:8] = W0
    t["lD0"] = _blockdiag(D0, 8)
    for i, W in [(1, W1), (2, W2)]:
        D = np.zeros((8, 8), dtype=np.float32)
        for k in range(8):
            for j in range(4):
                D[k, j] = W[WMAP[k], j]
                D[k, j + 4] = W[WMAP[k], j]
        t[f"lD{i}"] = _blockdiag(D, 16)
    PO = np.zeros((8, 4), dtype=np.float32)
    for k in range(8):
        PO[k, WMAP[k]] = 1.0
    t["lPO"] = _blockdiag(PO, 16)
    consts = np.zeros((128, 4), dtype=np.float32)
    for i, b in enumerate((b0, b1, b2)):
        consts[:, i] = np.tile(np.tile(np.asarray(b, np.float32), 2), 16)
    consts[:, 3] = np.tile([0., 0., 0., 0., PI2, PI2, PI2, PI2], 16)
    t["consts"] = consts
    for name in list(t):
        if name != "consts":
            t[name] = _to_bf16(t[name])
    return t


# ---------------- device kernel ----------------
MASK_A = [0, 0, 4, 4, 0, 0, 0, 0]
MASK_B = [3, 7, 3, 7, 0, 0, 0, 0]
W16_NAMES = ["lD0", "lD1", "lD2", "lA1_0", "lA2_0", "lA1_1", "lA2_1",
             "lA1_2", "lA2_2", "lPO"]
W_NAMES = W16_NAMES + ["consts"]
W_COLS = {"lD0": 64, "lD1": 128, "lD2": 128, "lA1_0": 128, "lA2_0": 128,
          "lA1_1": 128, "lA2_1": 128, "lA1_2": 128, "lA2_2": 128,
          "lPO": 64, "consts": 4}


def build_kernel(tc, x, out, wins):
    nc = tc.nc
    shufA = [8 * t_ + MASK_A[j] for t_ in range(4) for j in range(8)]
    shufB = [8 * t_ + MASK_B[j] for t_ in range(4) for j in range(8)]
    with contextlib.ExitStack() as ctx:
        wpool = ctx.enter_context(tc.tile_pool(name="w", bufs=1))
        dram = ctx.enter_context(tc.tile_pool(name="dram", bufs=4, space="DRAM"))
        xkp = ctx.enter_context(tc.tile_pool(name="xk", bufs=8))
        work = ctx.enter_context(tc.tile_pool(name="work", bufs=3))
        outp = ctx.enter_context(tc.tile_pool(name="outp", bufs=2))
        ps_mm = ctx.enter_context(tc.tile_pool(name="ps_mm", bufs=2, space="PSUM"))
        ps_po = ctx.enter_context(tc.tile_pool(name="ps_po", bufs=1, space="PSUM"))
        ps_ob = ctx.enter_context(tc.tile_pool(name="ps_ob", bufs=2, space="PSUM"))

        wt = {}
        for name in W16_NAMES:
            wtile = wpool.tile([128, W_COLS[name]], F16, tag=name)
            nc.sync.dma_start(wtile[:], wins[name][:, :])
            wt[name] = wtile
        ctile = wpool.tile([128, 4], F32, tag="consts")
        nc.sync.dma_start(ctile[:], wins["consts"][:, :])
        ident = wpool.tile([128, 128], F32, tag="ident")
        make_identity(nc, ident)
        lA1 = [wt["lA1_0"], wt["lA1_1"], wt["lA1_2"]]
        lA2 = [wt["lA2_0"], wt["lA2_1"], wt["lA2_2"]]
        lD = [None, wt["lD1"], wt["lD2"]]

        xin = x.rearrange("(ss k) f -> ss k f", ss=N_SS)
        # out row = ss*16384 + c*4096 + aa*1024 + ap*8 + r
        ov = out.rearrange("(ss c aa ap r) w -> ss aa ap c (r w)",
                           ss=N_SS, c=CH, aa=4, ap=128, r=8)

        # cast the whole input fp32->bf16 up-front; casts run on the SWDGE
        # queues and overlap compute after the first superstep
        x16s = []
        for ss in range(N_SS):
            x16 = dram.tile([16384, 16], F16, tag="x16")
            nc.gpsimd.dma_start(x16[:], xin[ss])
            x16s.append(x16)

        for ss in range(N_SS):
            x16v = x16s[ss][:].rearrange("(c a r) f -> c a (r f)", c=CH, a=512, r=8)
            xks = []
            for c in range(CH):
                xk = xkp.tile([128, 512], F16, tag="xk")
                nc.sync.dma_start(xk[:], x16v[c], transpose=True)
                xks.append(xk)

            pre = ps_mm.tile([128, 1024], F32, tag="mm")
            for c in range(CH):
                p0 = (c & 1) * 64
                c0 = (c >> 1) * 512
                nc.tensor.matmul(pre[p0:p0 + 64, c0:c0 + 512], wt["lD0"][:],
                                 xks[c][:], start=True, stop=True)

            vin = None
            for li in range(3):
                if li > 0:
                    pre = ps_mm.tile([128, 1024], F32, tag="mm")
                    for h in range(2):
                        nc.tensor.matmul(pre[:, h * 512:(h + 1) * 512], lD[li][:],
                                         vin[:, h * 512:(h + 1) * 512],
                                         start=True, stop=True)
                h8 = work.tile([128, 1024], F16, tag="h8")
                nc.scalar.activation(h8[:], pre[:], mybir.ActivationFunctionType.Tanh,
                                     bias=ctile[:, li:li + 1], scale=1.0)
                trig = work.tile([128, 1024], F16, tag="trig")
                nc.scalar.activation(trig[:], h8[:], mybir.ActivationFunctionType.Sin,
                                     bias=ctile[:, 3:4], scale=1.0)
                ga = work.tile([128, 1024], F16, tag="ga")
                gb = work.tile([128, 1024], F16, tag="gb")
                nc.vector.stream_shuffle(ga[:].bitcast(U32), trig[:].bitcast(U32), shufA)
                nc.vector.stream_shuffle(gb[:].bitcast(U32), trig[:].bitcast(U32), shufB)
                g = work.tile([128, 1024], F16, tag="g")
                nc.vector.tensor_mul(g[:], ga[:], gb[:])
                r1 = ps_mm.tile([128, 1024], F32, tag="mm")
                for h in range(2):
                    nc.tensor.matmul(r1[:, h * 512:(h + 1) * 512], lA1[li][:],
                                     g[:, h * 512:(h + 1) * 512], start=True, stop=False)
                for h in range(2):
                    nc.tensor.matmul(r1[:, h * 512:(h + 1) * 512], lA2[li][:],
                                     trig[:, h * 512:(h + 1) * 512], start=False, stop=True)
                v = work.tile([128, 1024], F16, tag="v")
                nc.vector.tensor_mul(v[:], trig[:], r1[:])
                vin = v

            po = ps_po.tile([64, 1024], F32, tag="po")
            for h in range(2):
                nc.tensor.matmul(po[:, h * 512:(h + 1) * 512], wt["lPO"][:],
                                 vin[:, h * 512:(h + 1) * 512], start=True, stop=True)
            so = outp.tile([64, 1024], F32, tag="so")
            nc.scalar.copy(so[:], po[:])
            ob = ps_ob.tile([128, 512], F32, tag="ob")
            for k in range(8):
                nc.tensor.transpose(ob[:, k * 64:(k + 1) * 64],
                                    so[:, k * 128:(k + 1) * 128], ident[0:64, 0:64])
            sob = outp.tile([128, 512], F32, tag="sob")
            nc.vector.tensor_copy(sob[:], ob[:])
            for k in range(8):
                chi, aa = k >> 2, k & 3
                eng = nc.sync if k % 2 == 0 else nc.scalar
                eng.dma_start(ov[ss, aa, :, 2 * chi:2 * chi + 2],
                              sob[:, k * 64:(k + 1) * 64])


# Force Tanh/Sin into a single resident ACT table set (silu_and_others holds
# both) so the table-load pass doesn't thrash between per-func sets. Dict
# order/indices are preserved so act_func_set_id stays consistent.
from concourse import hw_specs as _hw_specs
import concourse.bacc as _bacc_mod
_orig_get_tables = _hw_specs.get_activation_tables

def _patched_get_tables(arch):
    tabs = _orig_get_tables(arch)
    out = {}
    for name, s in tabs.items():
        s2 = set(s)
        if name != "silu_and_others":
            s2.discard(mybir.ActivationFunctionType.Tanh)
            s2.discard(mybir.ActivationFunctionType.Sin)
        out[name] = s2
    return out

_hw_specs.get_activation_tables = _patched_get_tables
for _mod in (_bacc_mod,):
    if hasattr(_mod, "get_activation_tables"):
        _mod.get_activation_tables = _patched_get_tables


_CACHE = {}


def _get_compiled():
    if "nc" in _CACHE:
        return _CACHE["nc"], _CACHE["tiles"]
    nc = bacc.Bacc("TRN2", target_bir_lowering=False, debug=False,
                   num_devices=N_CORES)
    x_ap = nc.dram_tensor("x", [B_CORE, D_IN], F32, kind="ExternalInput").ap()
    out_ap = nc.dram_tensor("out", [B_CORE, 4], F32, kind="ExternalOutput").ap()
    wins = {}
    for name in W16_NAMES:
        wins[name] = nc.dram_tensor(name, [128, W_COLS[name]], F16,
                                    kind="ExternalInput").ap()
    wins["consts"] = nc.dram_tensor("consts", [128, 4], F32,
                                    kind="ExternalInput").ap()
    with tile.TileContext(nc) as tc:
        build_kernel(tc, x_ap, out_ap, wins)
    nc.compile()
    _CACHE["nc"] = nc
    _CACHE["tiles"] = None
    return nc, None


def kernel(x, theta, W0, b0, W1, b1, W2, b2):
    x = np.ascontiguousarray(np.asarray(x, dtype=np.float32))
    wt = host_tensors(np.asarray(theta), np.asarray(W0), np.asarray(b0),
                      np.asarray(W1), np.asarray(b1), np.asarray(W2),
                      np.asarray(b2))
    nc, _ = _get_compiled()
    in_maps = []
    for c in range(N_CORES):
        m = {"x": np.ascontiguousarray(x[c * B_CORE:(c + 1) * B_CORE])}
        for name in W_NAMES:
            m[name] = wt[name]
        in_maps.append(m)
    res = run_bass_kernel_spmd(nc, in_maps, core_ids=list(range(N_CORES)))
    outs = [res.results[c]["out"] for c in range(N_CORES)]
    return np.concatenate(outs, axis=0).astype(np.float32)
